# revision 1
# baseline (speedup 1.0000x reference)
"""Trainium2 Bass kernel for y = x @ W^T + b  (4096x4096 @ 4096x4096 + 4096).

Sharding: data-parallel over batch. Core c gets x rows [c*512:(c+1)*512];
W and b are replicated. Each core computes yT_c = W @ x_c^T + b[:, None]
(output transposed, [4096, 512]) and the host reassembles
y = concat([yT_c.T for c in cores], axis=0). No collectives.

Per-core kernel (bf16 compute, fp32 accumulate in PSUM):
  - x_c: SWDGE DMA-cast f32->bf16 into SBUF natural (one 3D-AP DMA),
    transposed on the PE (128x128 transpose-mode blocks) into resident
    xT [128, 32, 512].
  - W: per 128-row slab, SWDGE DMA-cast f32->bf16, PE transpose-mode
    blocks -> PSUM (GK=8 blocks per bank), DVE eviction
    to wT [128, 32, 128]; then 32 accumulating matmuls (lhsT=wT block,
    rhs=xT block, N=512) into one PSUM bank (4 accum + 4 transpose banks).
  - PSUM eviction fused with bias add on ScalarE, DMA out on sync.

Measured (min of 3, whole-NEFF neuron-profile exec_time_ns): ~325 us,
relative error ~2.0e-3 vs the fp32 reference.
"""

import os
import sys

for _p in ("/opt/trn_rl_repo", "/opt/pypackages"):
    if _p not in sys.path and os.path.isdir(_p):
        sys.path.append(_p)

import numpy as np

import concourse.bass as bass
import concourse.tile as tile
from concourse import bacc, mybir
from concourse.bass_utils import run_bass_kernel_spmd

N_CORES = 8
BATCH = 4096
IN_F = 4096
OUT_F = 4096
P = 128
B = BATCH // N_CORES          # 512 batch rows per core
KT = IN_F // P                # 32 contraction tiles
OT = OUT_F // P               # 32 output-feature tiles

_F32 = mybir.dt.float32
_BF16 = mybir.dt.bfloat16

_compiled_nc = None


def _build():
    nc = bacc.Bacc("TRN2", target_bir_lowering=False, debug=False,
                   num_devices=N_CORES)

    x = nc.dram_tensor("x", [B, IN_F], _F32, kind="ExternalInput")
    w = nc.dram_tensor("weight", [OUT_F, IN_F], _F32, kind="ExternalInput")
    bias = nc.dram_tensor("bias", [OUT_F], _F32, kind="ExternalInput")
    out = nc.dram_tensor("out", [OUT_F, B], _F32, kind="ExternalOutput")

    from concourse.masks import make_identity

    GK = 8                     # k-tiles transposed per PSUM bank batch
    WH = 1                     # W cast split per slab
    BT = B // P                # 4 batch tiles
    with tile.TileContext(nc) as tc:
        with tc.tile_pool(name="const", bufs=1) as const, \
             tc.tile_pool(name="wnat", bufs=2 * WH) as wnat_pool, \
             tc.tile_pool(name="wt", bufs=3) as wt_pool, \
             tc.tile_pool(name="tpsum", bufs=4, space="PSUM") as tpsum_pool, \
             tc.tile_pool(name="psum", bufs=4, space="PSUM") as psum_pool, \
             tc.tile_pool(name="yout", bufs=3) as y_pool:

            ident = const.tile([P, P], _BF16)
            make_identity(nc, ident)
            ident32 = const.tile([32, 32], _F32)
            make_identity(nc, ident32)

            # ---- bias: one DMA to [32, 128], PE transpose -> [128, 32]
            b_nat = const.tile([32, P], _F32)
            nc.scalar.dma_start(out=b_nat[:],
                                in_=bias[:].rearrange("(a b) -> a b", b=P))
            b_ps = tpsum_pool.tile([P, 32], _F32, name="b_ps", tag="tps")
            nc.tensor.transpose(b_ps[:], b_nat[:], ident32[:])
            bias_sb = const.tile([P, OT], _F32)
            nc.vector.tensor_copy(out=bias_sb[:], in_=b_ps[:])

            # ---- x: DMA-cast to SBUF natural, PE-transpose to xT (resident)
            x_nat = const.tile([P, BT, IN_F], _BF16)
            nc.gpsimd.dma_start(
                out=x_nat[:],
                in_=x[:, :].rearrange("(bt p) i -> p bt i", p=P))
            xT = const.tile([P, KT, B], _BF16)
            for kt in range(KT):
                pst = tpsum_pool.tile([P, BT, P], _BF16, name=f"xps{kt}",
                                      tag="tps")
                for bt in range(BT):
                    nc.tensor.transpose(pst[:, bt, :],
                                        x_nat[:, bt, kt * P:(kt + 1) * P],
                                        ident[:])
                nc.vector.tensor_copy(out=xT[:, kt, :], in_=pst[:])

            # ---- main loop over output-feature tiles
            IH = IN_F // WH
            KH = IH // P   # k-tiles per half-slab
            for ot in range(OT):
                w_nat = [wnat_pool.tile([P, IH], _BF16, tag=f"wnat{h}",
                                        name=f"wnat{h}_{ot}")
                         for h in range(WH)]
                for h in range(WH):
                    nc.gpsimd.dma_start(
                        out=w_nat[h][:],
                        in_=w[ot * P:(ot + 1) * P, h * IH:(h + 1) * IH])

                # Transpose slab on the PE (transpose-mode), GK blocks per
                # PSUM bank, DVE-evicted per bank.
                wT = wt_pool.tile([P, KT, P], _BF16)
                for g in range(KT // GK):
                    pst = tpsum_pool.tile([P, GK, P], _BF16, tag="tps",
                                          name=f"wps_{ot}_{g}")
                    for j in range(GK):
                        kt = g * GK + j
                        src = w_nat[kt // KH]
                        k0 = (kt % KH) * P
                        nc.tensor.transpose(pst[:, j, :],
                                            src[:, k0:k0 + P],
                                            ident[:])
                    nc.vector.tensor_copy(out=wT[:, g * GK:(g + 1) * GK, :],
                                          in_=pst[:])

                ps = psum_pool.tile([P, B], _F32)
                for kt in range(KT):
                    nc.tensor.matmul(ps[:], lhsT=wT[:, kt, :],
                                     rhs=xT[:, kt, :],
                                     start=(kt == 0), stop=(kt == KT - 1))

                ysb = y_pool.tile([P, B], _F32)
                nc.scalar.activation(ysb[:], ps[:],
                                     mybir.ActivationFunctionType.Identity,
                                     bias=bias_sb[:, ot:ot + 1])
                nc.sync.dma_start(out=out[ot * P:(ot + 1) * P, :], in_=ysb[:])

    nc.compile()
    return nc


def _get_nc():
    global _compiled_nc
    if _compiled_nc is None:
        _compiled_nc = _build()
    return _compiled_nc


def _run(inputs, trace=False, trace_cores=None):
    x = np.ascontiguousarray(np.asarray(inputs["x"], dtype=np.float32))
    w = np.ascontiguousarray(np.asarray(inputs["weight"], dtype=np.float32))
    b = np.ascontiguousarray(np.asarray(inputs["bias"], dtype=np.float32))

    nc = _get_nc()
    in_maps = [
        {"x": x[c * B:(c + 1) * B], "weight": w, "bias": b}
        for c in range(N_CORES)
    ]
    res = run_bass_kernel_spmd(nc, in_maps, core_ids=list(range(N_CORES)),
                               trace=trace, trace_cores=trace_cores)
    y = np.concatenate([res.results[c]["out"].T for c in range(N_CORES)], axis=0)
    return y, res


def kernel(**inputs):
    y, _ = _run(inputs)
    return y



# revision 4
# speedup vs baseline: 1.2403x; 1.2403x over previous
"""Trainium2 Bass kernel for y = x @ W^T + b  (4096x4096 @ 4096x4096 + 4096).

Sharding: 2D (2 batch halves x 4 feature quarters). Core c = (bh, oq) gets
x^T[:, bh*2048:(bh+1)*2048] and W^T[:, oq*1024:(oq+1)*1024] (host-side
layout transposes, values untouched) and computes the natural-layout chunk
y[bh, oq] = x_bh @ W_oq^T + b_oq. Host reassembles the 2x4 grid.

Per-core kernel (bf16 compute, fp32 accumulate in PSUM):
  - W^T chunk [4096, 1024]: DMA-cast f32->bf16 into resident SBUF
    [128, 32, 1024], streamed kt-ascending (32 chunks).
  - x^T blocks [4096, 128]: DMA-cast f32->bf16 to [128, 32, 128], the
    matmul stationary operand.
  - Phase 1: 4 b-tiles x 2 psum banks accumulate kt-interleaved so the PE
    ramps with the W stream (8 matmuls per arriving W k-chunk).
  - Phase 2: remaining 12 b-tiles at full PE rate, x blocks prefetched.
  - Eviction: DVE tensor_tensor add with replicated bias, DMA out natural.
"""

import os
import sys

for _p in ("/opt/trn_rl_repo", "/opt/pypackages"):
    if _p not in sys.path and os.path.isdir(_p):
        sys.path.append(_p)

import numpy as np

import concourse.bass as bass
import concourse.tile as tile
from concourse import bacc, mybir
from concourse.bass_utils import run_bass_kernel_spmd

N_CORES = 8
BATCH = 4096
IN_F = 4096
OUT_F = 4096
P = 128
BH = 2                       # batch groups
OQ = 4                       # out-feature groups
B = BATCH // BH              # 2048 batch rows per core
O = OUT_F // OQ              # 1024 out features per core
KT = IN_F // P               # 32 contraction tiles
BT = B // P                  # 16 batch tiles per core
OS = O // 512                # 2 psum spans of 512
PH1 = 4                      # b-tiles accumulated in phase 1 (uses 8 banks)

_F32 = mybir.dt.float32
_BF16 = mybir.dt.bfloat16

_compiled_nc = None


def _build():
    nc = bacc.Bacc("TRN2", target_bir_lowering=False, debug=False,
                   num_devices=N_CORES)

    xt = nc.dram_tensor("xt", [IN_F, B], _F32, kind="ExternalInput")
    wt = nc.dram_tensor("wt", [IN_F, O], _F32, kind="ExternalInput")
    bias = nc.dram_tensor("bias", [P, O], _F32, kind="ExternalInput")
    out = nc.dram_tensor("out", [B, O], _F32, kind="ExternalOutput")

    with tile.TileContext(nc) as tc:
        with tc.tile_pool(name="const", bufs=1) as const, \
             tc.tile_pool(name="xblk", bufs=8) as x_pool, \
             tc.tile_pool(name="psum", bufs=8, space="PSUM") as psum_pool, \
             tc.tile_pool(name="yout", bufs=3) as y_pool:

            bias_sb = const.tile([P, O], _F32)
            nc.scalar.dma_start(out=bias_sb[:], in_=bias[:, :])

            # Resident W^T chunk ([128, kt, O], kt-ascending) and x^T
            # stationary blocks all stream through the gpsimd (SWDGE)
            # queue — the only engine that can DMA-cast f32->bf16. The
            # dispatch order interleaves phase-1 x blocks with the first
            # W chunks so the PE can ramp with both streams; the x pool
            # depth throttles the phase-2 prefetch distance.
            wsb = const.tile([P, KT, O], _BF16)
            xblk = [x_pool.tile([P, KT, P], _BF16, name=f"xblk{bt}", tag="xblk")
                    for bt in range(BT)]
            XSUB = 4             # sub-DMAs per phase-1 block for pacing
            KS = KT // XSUB

            def w_chunk(kt):
                nc.gpsimd.dma_start(out=wsb[:, kt, :],
                                    in_=wt[kt * P:(kt + 1) * P, :])

            def x_sub(bt, s, ks):
                src = xt[:, bt * P:(bt + 1) * P].rearrange(
                    "(kt p) b -> p kt b", p=P)
                nc.gpsimd.dma_start(
                    out=xblk[bt][:, s * ks:(s + 1) * ks, :],
                    in_=src[:, s * ks:(s + 1) * ks, :])

            for kt in range(KT):
                if kt < PH1 * XSUB:
                    bt, s = divmod(kt, XSUB)
                    x_sub(bt, s, KS)
                w_chunk(kt)
            for bt in range(PH1, BT):
                x_sub(bt, 0, KT)

            def evict(bt, ps):
                ysb = y_pool.tile([P, O], _F32, name=f"y{bt}", tag="y")
                for osp in range(OS):
                    nc.vector.tensor_tensor(
                        ysb[:, osp * 512:(osp + 1) * 512],
                        ps[osp][:],
                        bias_sb[:, osp * 512:(osp + 1) * 512],
                        mybir.AluOpType.add)
                nc.sync.dma_start(out=out[bt * P:(bt + 1) * P, :], in_=ysb[:])

            # ---- phase 1: 4 b-tiles, kt-interleaved with the W stream
            ps1 = [[psum_pool.tile([P, 512], _F32, name=f"ps1_{bt}_{osp}", tag="ps")
                    for osp in range(OS)] for bt in range(PH1)]
            for kt in range(KT):
                for bt in range(PH1):
                    for osp in range(OS):
                        nc.tensor.matmul(
                            ps1[bt][osp][:],
                            lhsT=xblk[bt][:, kt, :],
                            rhs=wsb[:, kt, osp * 512:(osp + 1) * 512],
                            start=(kt == 0), stop=(kt == KT - 1))
            for bt in range(PH1):
                evict(bt, ps1[bt])

            # ---- phase 2: remaining b-tiles at full PE rate
            for bt in range(PH1, BT):
                ps = [psum_pool.tile([P, 512], _F32, name=f"ps2_{bt}_{osp}", tag="ps")
                      for osp in range(OS)]
                for kt in range(KT):
                    for osp in range(OS):
                        nc.tensor.matmul(
                            ps[osp][:],
                            lhsT=xblk[bt][:, kt, :],
                            rhs=wsb[:, kt, osp * 512:(osp + 1) * 512],
                            start=(kt == 0), stop=(kt == KT - 1))
                evict(bt, ps)

    nc.compile()
    return nc


def _get_nc():
    global _compiled_nc
    if _compiled_nc is None:
        _compiled_nc = _build()
    return _compiled_nc


def _run(inputs, trace=False, trace_cores=None):
    x = np.asarray(inputs["x"], dtype=np.float32)
    w = np.asarray(inputs["weight"], dtype=np.float32)
    b = np.asarray(inputs["bias"], dtype=np.float32)

    nc = _get_nc()
    in_maps = []
    for c in range(N_CORES):
        bh, oq = divmod(c, OQ)
        xt_c = np.ascontiguousarray(x[bh * B:(bh + 1) * B, :].T)
        wt_c = np.ascontiguousarray(w[oq * O:(oq + 1) * O, :].T)
        bias_c = np.ascontiguousarray(
            np.broadcast_to(b[oq * O:(oq + 1) * O], (P, O)))
        in_maps.append({"xt": xt_c, "wt": wt_c, "bias": bias_c})

    res = run_bass_kernel_spmd(nc, in_maps, core_ids=list(range(N_CORES)),
                               trace=trace, trace_cores=trace_cores)
    y = np.empty((BATCH, OUT_F), dtype=np.float32)
    for c in range(N_CORES):
        bh, oq = divmod(c, OQ)
        y[bh * B:(bh + 1) * B, oq * O:(oq + 1) * O] = res.results[c]["out"]
    return y, res


def kernel(**inputs):
    y, _ = _run(inputs)
    return y
